# revision 30
# baseline (speedup 1.0000x reference)
"""Trainium2 Bass kernel for MultiHeadAttention with relative position bias.

Reference computation (B=2, S=2048, D=1024, H=16, Dk=64, MAX_REL=128):
    Q,K,V = x@W{q,k,v}.T + b      (per-head reshape)
    scores = QK^T/sqrt(Dk) + rel_bias_matrix
    out = softmax(scores) @ V, heads merged, @ Wo.T + bo

Sharding (8 cores): core c handles batch b=c//4 and 4 heads hg=4*(c%4)..+4
(data + head parallel). Q/K/V projections column-split per head group,
Wo row-split; the partial outputs are summed on the host (the "all-reduce").

Per-core device algorithm (all matmul operands bf16; f32 PSUM accumulate):
  xT (1024,2048) -> Q^T,K^T (c_local=256, S) on PE; V as (S, 256).
  Attention runs in 8 blocks = (head pair, 512-wide q window), 16 k-chunks
  each.  Per k-chunk one [128,1024] PSUM tile holds BOTH heads' scores
  (row-tiled 64x128 QK matmuls write bank-aligned halves), so ONE exp
  activation covers the pair: the scalar engine -- the hard wall at
  4 heads x S^2 = 16.8M exps/core (~110us) -- runs at full [128,1024]
  efficiency while accs shrink to one PSUM bank each.
  The "future" region (q-k <= -128) is fixed with a constant multiply and
  the 384-wide Toeplitz band with a host-precomputed exp(bias - c_past)
  tile (both DVE, bf16).  P^T stored bf16.
  PV: lhsT is the M=128 augmented [V_h0 | ones] (hh=0) / [ones | V_h1]
  (hh=1), so acc rows carry the head output AND the softmax denominator
  replicated across 64 partitions (PV matmuls are N-bound, so this is
  free).  Normalize: two partition-shifting PSUM->SBUF copies collect both
  heads' denominators, reciprocal_approx_fast (DVE), then two fused
  evict-normalize multiplies into bf16 ct.
  Wo partial = ct.T @ (Wo^T rows), per-q normalization already applied.

PSUM (8 banks): score ring 2x[128,1024] (4) + accs 2x[128,512] (2) +
filler ring 2x[128,512] (2).  All projection/Wo work is split into
self-contained "pieces" (~0.5-0.9us: own filler slot, matmuls, evict) and
interleaved one-per-k-chunk into the attention rounds, riding the PE's
true slack under the exp stream instead of serializing against it.
Output is written bf16 (summed f32 on the host).
"""

import math
import os
import sys

for _p in ("/opt/trn_rl_repo", "/root/.axon_site", "/root/.axon_site/_ro/trn_rl_repo",
           "/root/.axon_site/_ro/pypackages"):
    if os.path.isdir(_p) and _p not in sys.path:
        sys.path.append(_p)

import numpy as np
import ml_dtypes

import concourse.bass as bass
import concourse.mybir as mybir
import concourse.tile as tile
from concourse import bacc
from contextlib import ExitStack

# Problem constants (hardcoded per the contract).
B, S, D = 2, 2048, 1024
H, DK = 16, 64
MAX_REL = 128
N_CORES = 8
CORES_PER_BATCH = 4
HEADS_PER_CORE = H // CORES_PER_BATCH  # 4
CL = HEADS_PER_CORE * DK               # 256 local channels
N_PAIRS = HEADS_PER_CORE // 2          # 2 head pairs
QW = 512                               # q window per attention block
N_QW = S // QW                         # 4
N_KC = S // 128                        # 16 k chunks
BAND = 3 * 128                         # band width in q for one k chunk
NDC = D // 128                         # 8 contraction chunks

F32 = mybir.dt.float32
BF16 = mybir.dt.bfloat16

SCALE = 1.0 / math.sqrt(DK)

EXP = mybir.ActivationFunctionType.Exp


def build_program(reps=1):
    nc = bacc.Bacc("TRN2", target_bir_lowering=False, debug=False)

    xt_d = nc.declare_dram_parameter("xt", [D, S], BF16, isOutput=False)
    wqt_d = nc.declare_dram_parameter("wqt", [D, CL], BF16, isOutput=False)
    wkt_d = nc.declare_dram_parameter("wkt", [D, CL], BF16, isOutput=False)
    wvt_d = nc.declare_dram_parameter("wvt", [D, CL], BF16, isOutput=False)
    wot_d = nc.declare_dram_parameter("wot", [CL, D], BF16, isOutput=False)
    bqk_d = nc.declare_dram_parameter("bqk", [128, 4], F32, isOutput=False)
    band_d = nc.declare_dram_parameter("band", [128, HEADS_PER_CORE, BAND], BF16,
                                       isOutput=False)
    # per-head activation constants, replicated over partitions:
    # [:, 2h] = exp(c_fut - c_past) multiplier
    abias_d = nc.declare_dram_parameter("abias", [128, 2 * HEADS_PER_CORE], F32,
                                        isOutput=False)
    out_d = nc.declare_dram_parameter("out_p", [S, D], BF16, isOutput=True)

    with tile.TileContext(nc) as tc, ExitStack() as ctx:
        # ---------- long-lived SBUF ----------
        persist = ctx.enter_context(tc.tile_pool(name="persist", bufs=1))
        q_sb = persist.tile([128, 2, S], BF16, tag="q_sb")
        k_sb = persist.tile([128, 2, S], BF16, tag="k_sb")
        # per (kc, pair): [V_h0(64) | ones(128) | V_h1(64)]
        v_sb = persist.tile([128, N_KC, N_PAIRS, 256], BF16, tag="v_sb")
        ct_sb = persist.tile([128, 2, S], BF16, tag="ct_sb")
        wo_sb = persist.tile([128, 2, D], BF16, tag="wo_sb")
        band_sb = persist.tile([128, HEADS_PER_CORE, BAND], BF16, tag="band_sb")
        bqk_sb = persist.tile([128, 4], F32, tag="bqk_sb")
        abias_sb = persist.tile([128, 2 * HEADS_PER_CORE], F32, tag="abias_sb")

        xw = ctx.enter_context(tc.tile_pool(name="xw", bufs=1))
        xt_sb = xw.tile([128, NDC, S], BF16, tag="xt_sb")
        wq_sb = xw.tile([128, NDC, CL], BF16, tag="wq_sb")
        wk_sb = xw.tile([128, NDC, CL], BF16, tag="wk_sb")
        wv_sb = xw.tile([128, NDC, CL], BF16, tag="wv_sb")

        # ---------- PSUM pools: 4 + 2 + 2 banks ----------
        stp = ctx.enter_context(tc.tile_pool(name="stp", bufs=2, space="PSUM"))
        accp = ctx.enter_context(tc.tile_pool(name="accp", bufs=2, space="PSUM"))
        fillp = ctx.enter_context(tc.tile_pool(name="fillp", bufs=2, space="PSUM"))

        # ---------- small pools ----------
        outp = ctx.enter_context(tc.tile_pool(name="outp", bufs=4))
        nrm = ctx.enter_context(tc.tile_pool(name="nrm", bufs=4))
        ptp = ctx.enter_context(tc.tile_pool(name="ptp", bufs=8))

        sb = dict(q=q_sb, k=k_sb, v=v_sb, ct=ct_sb, wo=wo_sb, band=band_sb,
                  bqk=bqk_sb, abias=abias_sb, xt=xt_sb, wq=wq_sb, wk=wk_sb,
                  wv=wv_sb)
        dram = dict(xt=xt_d, wqt=wqt_d, wkt=wkt_d, wvt=wvt_d, wot=wot_d,
                    bqk=bqk_d, band=band_d, abias=abias_d, out=out_d)
        pools = dict(stp=stp, accp=accp, fillp=fillp, outp=outp, nrm=nrm,
                     ptp=ptp)

        for rep in range(reps):
            _phases(nc, tc, sb, dram, pools, rep)

    nc.compile()
    return nc


def _phases(nc, tc, sb, dram, pools, rep):
    q_sb, k_sb, v_sb, ct_sb, wo_sb = sb["q"], sb["k"], sb["v"], sb["ct"], sb["wo"]
    band_sb, bqk_sb, abias_sb = sb["band"], sb["bqk"], sb["abias"]
    xt_sb, wq_sb, wk_sb, wv_sb = sb["xt"], sb["wq"], sb["wk"], sb["wv"]
    stp, accp, fillp, outp, nrm, ptp = (pools[n] for n in
                                        ("stp", "accp", "fillp", "outp",
                                         "nrm", "ptp"))

    xt_v = dram["xt"].ap().rearrange("(c p) s -> p c s", p=128)

    # ---------- input DMAs (two HWDGE queues share HBM; order by need) -----
    q2 = (nc.sync, nc.scalar)
    nc.sync.dma_start(out=wk_sb, in_=dram["wkt"].ap().rearrange("(c p) m -> p c m", p=128))
    nc.scalar.dma_start(out=bqk_sb, in_=dram["bqk"].ap())
    for dc in range(NDC):
        q2[dc % 2].dma_start(out=xt_sb[:, dc, :], in_=xt_v[:, dc, :])
    nc.scalar.dma_start(out=wq_sb, in_=dram["wqt"].ap().rearrange("(c p) m -> p c m", p=128))
    nc.sync.dma_start(out=wv_sb, in_=dram["wvt"].ap().rearrange("(c p) m -> p c m", p=128))
    nc.scalar.dma_start(out=wo_sb, in_=dram["wot"].ap().rearrange("(c p) m -> p c m", p=128))
    nc.sync.dma_start(out=abias_sb, in_=dram["abias"].ap())
    nc.sync.dma_start(out=band_sb, in_=dram["band"].ap())
    # ones blocks of the augmented V (middle 128 columns of each pair block)
    nc.vector.memset(v_sb[:, :, :, 64:192], 1.0)

    # ---------- self-contained filler pieces (slot + matmuls + evict) ------
    def piece_qk(w_sb, o_sb, bcol, j, quarter):
        """One [128,256] slice of a Q/K projection (~0.9us of PE)."""
        slot = fillp.tile([128, 512], F32, tag="fill", name="pq_slot")
        c0 = quarter * 256
        for dc in range(NDC):
            nc.tensor.matmul(
                slot[:, 0:256],
                lhsT=w_sb[:, dc, j * 128:(j + 1) * 128],
                rhs=xt_sb[:, dc, c0:c0 + 256],
                start=(dc == 0), stop=(dc == NDC - 1),
            )
        nc.vector.tensor_scalar_add(
            out=o_sb[:, j, c0:c0 + 256],
            in0=slot[:, 0:256],
            scalar1=bqk_sb[:, bcol + j:bcol + j + 1],
        )

    def piece_v(sc):
        """V projection for one s-chunk: 8 matmuls + 2 DVE evicts."""
        slot = fillp.tile([128, 512], F32, tag="fill", name="pv_slot")
        for dc in range(NDC):
            nc.tensor.matmul(
                slot[:, 0:CL],
                lhsT=xt_sb[:, dc, sc * 128:(sc + 1) * 128],
                rhs=wv_sb[:, dc, :],
                start=(dc == 0), stop=(dc == NDC - 1),
            )
        src = slot[:, 0:CL].rearrange("p (hp dd) -> p hp dd", hp=2)
        # even heads -> cols 0:64, odd heads -> cols 192:256 of pair block
        nc.vector.tensor_copy(out=v_sb[:, sc, :, 0:64], in_=src[:, :, 0:64])
        nc.vector.tensor_copy(out=v_sb[:, sc, :, 192:256], in_=src[:, :, 64:128])

    def piece_wo(st_i, mt, dma_eng, evict_eng=None):
        """Half a Wo chunk: [128,512] of out rows st_i*128.. (~0.6us PE)."""
        ps = fillp.tile([128, 512], F32, tag="fill", name="wo_ps")
        o_sb = outp.tile([128, 512], BF16, tag="o_sb", name="o_sb")
        for j in range(2):
            nc.tensor.matmul(
                ps[:, 0:512],
                lhsT=ct_sb[:, j, st_i * 128:(st_i + 1) * 128],
                rhs=wo_sb[:, j, mt * 512:(mt + 1) * 512],
                start=(j == 0), stop=(j == 1),
            )
        if evict_eng is None:
            nc.vector.tensor_copy(out=o_sb, in_=ps)
        else:
            evict_eng.copy(out=o_sb, in_=ps)
        dma_eng.dma_start(
            out=dram["out"].ap()[st_i * 128:(st_i + 1) * 128,
                                 mt * 512:(mt + 1) * 512],
            in_=o_sb)

    # ---------- attention ----------
    def fixups(pair, hh, kc, w0, pt_dst):
        """Band/future multiplicative fixups on one head's P^T window."""
        k0 = kc * 128
        h = 2 * pair + hh
        # future region (q <= k0-129): multiply by exp(c_fut - c_past)
        fut_end = min(max(k0 - 128, w0), w0 + QW)
        n_fut = fut_end - w0
        if n_fut > 0:
            nc.vector.tensor_scalar_mul(
                out=pt_dst[:, 0:n_fut], in0=pt_dst[:, 0:n_fut],
                scalar1=abias_sb[:, 2 * h:2 * h + 1],
            )
        # band: q in [k0-128, k0+256) -> multiply exp(bias - c_past)
        b_lo = max(k0 - 128, w0)
        b_hi = min(k0 + 2 * 128, w0 + QW)
        if b_hi > b_lo:
            m0 = b_lo - (k0 - 128)
            nc.vector.tensor_mul(
                out=pt_dst[:, b_lo - w0:b_hi - w0],
                in0=pt_dst[:, b_lo - w0:b_hi - w0],
                in1=band_sb[:, h, m0:m0 + (b_hi - b_lo)],
            )

    def qk_round(pair, kc, w0):
        """Both heads' scores into one [128,1024] PSUM tile, ONE exp."""
        k0 = kc * 128
        st = stp.tile([128, 1024], F32, tag="st", name="st")
        for hh in range(2):
            p0 = hh * 64
            nc.tensor.matmul(
                st[:, hh * 512:(hh + 1) * 512],
                lhsT=k_sb[p0:p0 + 64, pair, k0:k0 + 128],
                rhs=q_sb[p0:p0 + 64, pair, w0:w0 + QW],
                start=True, stop=True,
                tile_position=(p0, 0),
            )
        pt = ptp.tile([128, 2 * QW], BF16, tag="pt", name="pt")
        nc.scalar.activation(out=pt, in_=st, func=EXP, scale=SCALE)
        for hh in range(2):
            fixups(pair, hh, kc, w0, pt[:, hh * QW:(hh + 1) * QW])
        return pt

    def pv_round(pair, kc, pt, accs):
        for hh in range(2):
            nc.tensor.matmul(
                accs[hh][:, 0:QW],
                lhsT=v_sb[:, kc, pair, hh * 128:(hh + 1) * 128],
                rhs=pt[:, hh * QW:(hh + 1) * QW],
                start=(kc == 0), stop=(kc == N_KC - 1),
            )

    def normalize(pair, w0, acc_a, acc_b):
        """Shift-copy denominators, approx-recip, fused evict-normalize.

        acc_a (hh=0) rows: 0:64 = ct_h0 unnormalized, 64:128 = den_h0 x64.
        acc_b (hh=1) rows: 0:64 = den_h1 x64, 64:128 = ct_h1 unnormalized.
        """
        den = nrm.tile([128, QW], F32, tag="den", name="den")
        rden = nrm.tile([128, QW], F32, tag="rden", name="rden")
        nc.vector.tensor_copy(out=den[0:64, :], in_=acc_a[64:128, :])
        nc.vector.tensor_copy(out=den[64:128, :], in_=acc_b[0:64, :])
        nc.vector.reciprocal_approx_fast(out=rden, in_=den)
        nc.vector.tensor_mul(
            out=ct_sb[0:64, pair, w0:w0 + QW],
            in0=acc_a[0:64, :], in1=rden[0:64, :],
        )
        nc.vector.tensor_mul(
            out=ct_sb[64:128, pair, w0:w0 + QW],
            in0=acc_b[64:128, :], in1=rden[64:128, :],
        )

    def attn_block(pair, qw, fillers=(), defer=1):
        """One (head pair, q window): 16 QK+exp rounds, QK emitted two rounds
        ahead of the fillers/PV so the in-order PE queue always has the next
        scores ready for the exp stream; PV lags `defer` rounds."""
        w0 = qw * QW
        acc_a = accp.tile([128, QW], F32, tag="acc", name="acc_a")
        acc_b = accp.tile([128, QW], F32, tag="acc", name="acc_b")
        accs = [acc_a, acc_b]
        pts = {0: qk_round(pair, 0, w0), 1: qk_round(pair, 1, w0)}
        for kc in range(N_KC):
            if kc < len(fillers):
                for f in fillers[kc]:
                    f()
            if kc >= defer:
                pv_round(pair, kc - defer, pts.pop(kc - defer), accs)
            if kc + 2 < N_KC:
                pts[kc + 2] = qk_round(pair, kc + 2, w0)
        for k2 in sorted(pts):
            pv_round(pair, k2, pts[k2], accs)
        normalize(pair, w0, acc_a, acc_b)

    # ---------- schedule ----------
    # Pieces: K/Q quarters cover 256 columns of S; window w needs Q quarters
    # 2w, 2w+1; k-chunk kc needs K quarter kc//2.  Blocks run pair-major so
    # pair 1's projections spread across pair 0's four blocks.
    K = lambda j, qq: (lambda: piece_qk(wk_sb, k_sb, 2, j, qq))
    Q = lambda j, qq: (lambda: piece_qk(wq_sb, q_sb, 0, j, qq))
    V = lambda sc: (lambda: piece_v(sc))
    W = lambda i, mt: (lambda: piece_wo(i, mt, nc.sync))

    # HAM warm-up: ~28 matmuls on a zeroed scratch tile run during the input
    # DMA window (no data deps), holding the PE clock at 8/8 so the real
    # projection pieces start warm.  Output is discarded.
    scratch = outp.tile([128, 512], BF16, tag="o_sb", name="scratch")
    nc.vector.memset(scratch, 0.0)
    junk = fillp.tile([128, 512], F32, tag="fill", name="junk")
    for _ in range(28):
        nc.tensor.matmul(junk[:, 0:512], lhsT=scratch[:, 0:128],
                         rhs=scratch[:, 0:512], start=True, stop=True)

    # Pre-attention flight: pair 0's K for kc0-7 and Q for window 0, six
    # 256-wide accumulations packed into 4 PSUM slots (bank-aligned pairs),
    # contraction-chunk-major so the matmuls ride the xt DMA arrivals.
    pre_s0 = stp.tile([128, 1024], F32, tag="st", name="pre_s0")
    pre_s1 = stp.tile([128, 1024], F32, tag="st", name="pre_s1")
    pre_f0 = fillp.tile([128, 512], F32, tag="fill", name="pre_f0")
    pre_f1 = fillp.tile([128, 512], F32, tag="fill", name="pre_f1")
    pre = [(pre_s0, 0, wk_sb, k_sb, 2, 0), (pre_s0, 512, wk_sb, k_sb, 2, 1),
           (pre_s1, 0, wk_sb, k_sb, 2, 2), (pre_s1, 512, wk_sb, k_sb, 2, 3),
           (pre_f0, 0, wq_sb, q_sb, 0, 0), (pre_f1, 0, wq_sb, q_sb, 0, 1)]
    for dc in range(NDC):
        for slot, off, w_sb, _, _, qq in pre:
            nc.tensor.matmul(
                slot[:, off:off + 256],
                lhsT=w_sb[:, dc, 0:128],
                rhs=xt_sb[:, dc, qq * 256:(qq + 1) * 256],
                start=(dc == 0), stop=(dc == NDC - 1),
            )
    for slot, off, _, o_sb, bcol, qq in pre:
        nc.vector.tensor_scalar_add(
            out=o_sb[:, 0, qq * 256:(qq + 1) * 256],
            in0=slot[:, off:off + 256],
            scalar1=bqk_sb[:, bcol:bcol + 1],
        )

    # B1 (0,w0): V (its own PV consumes it), K(0) kc8-15, Q(0) window 1
    fills = [
        [[V(0)], [V(1), K(0, 4)], [V(2)], [V(3), K(0, 5)],
         [V(4)], [V(5), K(0, 6)], [V(6)], [V(7), K(0, 7)],
         [V(8)], [V(9), Q(0, 2)], [V(10)], [V(11), Q(0, 3)],
         [V(12)], [V(13)], [V(14)], [V(15)]],
        # B2 (0,w1): Q(0) window 2, pair 1's K for kc0-7 (used from B5)
        [[Q(0, 4)], [], [K(1, 0)], [], [Q(0, 5)], [], [K(1, 1)], [],
         [K(1, 2)], [], [K(1, 3)], []],
        # B3 (0,w2): Q(0) window 3, pair 1's K for kc8-15
        [[Q(0, 6)], [], [K(1, 4)], [], [Q(0, 7)], [], [K(1, 5)], [],
         [K(1, 6)], [], [K(1, 7)], []],
        # B4 (0,w3): pair 1's Q windows 0,1
        [[Q(1, 0)], [], [Q(1, 1)], [], [Q(1, 2)], [], [Q(1, 3)], []],
        # B5 (1,w0): pair 1's Q windows 2,3
        [[Q(1, 4)], [], [Q(1, 5)], [], [Q(1, 6)], [], [Q(1, 7)], []],
        # B6 (1,w1): Wo for window 0 (both pairs' ct ready after B5)
        [[W(0, 0)], [], [W(0, 1)], [], [W(1, 0)], [], [W(1, 1)], [],
         [W(2, 0)], [], [W(2, 1)], [], [W(3, 0)], [], [W(3, 1)], []],
        # B7 (1,w2): Wo for window 1
        [[W(4, 0)], [], [W(4, 1)], [], [W(5, 0)], [], [W(5, 1)], [],
         [W(6, 0)], [], [W(6, 1)], [], [W(7, 0)], [], [W(7, 1)], []],
        # B8 (1,w3): Wo for window 2
        [[W(8, 0)], [], [W(8, 1)], [], [W(9, 0)], [], [W(9, 1)], [],
         [W(10, 0)], [], [W(10, 1)], [], [W(11, 0)], [], [W(11, 1)], []],
    ]
    bi = 0
    for pair in range(2):
        for qw in range(N_QW):
            attn_block(pair, qw, fills[bi])
            bi += 1
    # Keep the PE clock warm through the last block's normalize (DVE-only)
    # so the tail Wo matmuls run at full rate.
    junk2 = fillp.tile([128, 512], F32, tag="fill", name="junk2")
    for _ in range(12):
        nc.tensor.matmul(junk2[:, 0:512], lhsT=scratch[:, 0:128],
                         rhs=scratch[:, 0:512], start=True, stop=True)
    # Wo tail: window 3 (q rows 1536:2048); evicts alternate ACT/DVE and
    # the output DMAs alternate between both hardware queues.
    for i, (st_i, mt) in enumerate([(i, m) for i in range(12, 16)
                                    for m in range(2)]):
        piece_wo(st_i, mt, (nc.sync, nc.scalar)[i % 2],
                 evict_eng=(None, nc.scalar)[i % 2])


def make_core_inputs(x, Wq, bq, Wk, bk, Wv, bv, Wo, bo, rel_bias):
    """Host-side shard prep. Returns list of 8 in_maps."""
    bf16 = ml_dtypes.bfloat16
    x = np.asarray(x, np.float32)
    in_maps = []
    WqT = np.ascontiguousarray(np.asarray(Wq, np.float32).T)
    WkT = np.ascontiguousarray(np.asarray(Wk, np.float32).T)
    WvT = np.ascontiguousarray(np.asarray(Wv, np.float32).T)
    WoT = np.ascontiguousarray(np.asarray(Wo, np.float32).T)
    rel = np.asarray(rel_bias, np.float32)
    xt = [np.ascontiguousarray(x[b].T).astype(bf16) for b in range(B)]

    # band multiplier: [p, h_local, m] = exp(bias(q,k) - c_past), q-k = m-128-p
    p_i = np.arange(128)[:, None]
    m_i = np.arange(BAND)[None, :]
    delta = np.clip(m_i - 128 - p_i, -MAX_REL, MAX_REL) + MAX_REL  # [128, 384]

    for c in range(N_CORES):
        b = c // CORES_PER_BATCH
        g = c % CORES_PER_BATCH
        c0 = g * CL
        heads = np.arange(g * HEADS_PER_CORE, (g + 1) * HEADS_PER_CORE)

        bqk = np.empty((128, 4), np.float32)
        bqk[:, 0] = np.asarray(bq, np.float32)[c0:c0 + 128]
        bqk[:, 1] = np.asarray(bq, np.float32)[c0 + 128:c0 + 256]
        bqk[:, 2] = np.asarray(bk, np.float32)[c0:c0 + 128]
        bqk[:, 3] = np.asarray(bk, np.float32)[c0 + 128:c0 + 256]

        band = np.empty((128, HEADS_PER_CORE, BAND), np.float32)
        abias = np.empty((128, 2 * HEADS_PER_CORE), np.float32)
        for i, hg in enumerate(heads):
            c_past = rel[hg, 2 * MAX_REL]
            band[:, i, :] = np.exp(rel[hg][delta] - c_past)
            abias[:, 2 * i] = np.exp(rel[hg, 0] - c_past)  # future multiplier
            abias[:, 2 * i + 1] = c_past
        in_maps.append({
            "xt": xt[b],
            "wqt": np.ascontiguousarray(WqT[:, c0:c0 + CL]).astype(bf16),
            "wkt": np.ascontiguousarray(WkT[:, c0:c0 + CL]).astype(bf16),
            "wvt": np.ascontiguousarray(WvT[:, c0:c0 + CL]).astype(bf16),
            "wot": np.ascontiguousarray(WoT[c0:c0 + CL, :]).astype(bf16),
            "bqk": bqk,
            "band": band.astype(bf16),
            "abias": abias,
        })
    return in_maps


_NC_CACHE = {}


def get_program(**kw):
    key = tuple(sorted(kw.items()))
    if key not in _NC_CACHE:
        _NC_CACHE[key] = build_program(**kw)
    return _NC_CACHE[key]


def kernel(x, Wq, bq, Wk, bk, Wv, bv, Wo, bo, rel_bias):
    from concourse.bass_utils import run_bass_kernel_spmd

    nc = get_program()
    in_maps = make_core_inputs(x, Wq, bq, Wk, bk, Wv, bv, Wo, bo, rel_bias)
    res = run_bass_kernel_spmd(nc, in_maps, core_ids=list(range(N_CORES)))
    results = res.results

    Wo_np = np.asarray(Wo, np.float32)
    const = np.asarray(bv, np.float32) @ Wo_np.T + np.asarray(bo, np.float32)
    out = np.zeros((B, S, D), np.float32)
    for c in range(N_CORES):
        out[c // CORES_PER_BATCH] += np.asarray(results[c]["out_p"], np.float32)
    out += const[None, None, :]
    return out
